# revision 7
# baseline (speedup 1.0000x reference)
"""Combined contrastive + cross-entropy loss on 8 Trainium2 NeuronCores.

Math (matches the jax reference):
  d2[i,j]   = ||z_i||^2 + ||z_j||^2 - 2 z_i.z_j + 2*eps*(s_i - s_j) + D*eps^2
  dist      = sqrt(max(d2, 0))           (floor 1e-12 only matters on the
                                          diagonal, handled analytically)
  pair_loss = (1-same)*d2 + same*relu(1 - dist)^2
  contrastive = sum_{i<j} pair_loss / (N(N-1)/2)
  supervised  = mean CE(preds, labels)

Sharding: data-parallel over rows. Core c computes the 512 x 4096 block-row
of the pair matrix (its local latents against all latents) plus CE over its
512 rows of preds, and returns per-row-tile partial sums. The host sums the
8 x [128,32] / [128,4] partials in float64 and applies the closed-form
diagonal correction (sum over ALL (i,j) minus diagonal, halved == sum i<j).

Device tricks:
  * -2*G + c_n in one PSUM accumulation group: two K=128 matmuls with the
    transposed latents scaled by -2 on the host would change |values|;
    instead the -2 is folded into the column-constant row c_n's matmul
    partner and the gram matmuls use scale -2 via lhsT preprocessing -- see
    below: we pass zlocT pre-scaled by -2, so P = (-2 z_loc) . z_all + c_n.
  * per-row constant b_m = ||z_m||^2 + 2 eps s_m enters as the activation
    bias (per-partition scalar) of the Relu that clamps d2.
  * label equality mask: (lab_i - lab_j)^2 via a K=3 matmul of integer-exact
    f32 rows; used directly as the nonzero-predicate of copy_predicated.
  * pair contribution: sel = (labels differ) ? dist : relu(1-dist), then
    sel^2 row-reduced in one tensor_tensor_reduce.
  * CE: exp with per-partition bias (-rowmax) and fused row-sum accum_out;
    label pick via the same K=3 matmul trick against an iota table.
"""

import numpy as np

_N, _D, _C = 4096, 256, 1000
_NCORES = 8
_ROWS = _N // _NCORES          # 512 rows per core
_RB = _ROWS // 128             # 4 row tiles per core
_CT = _N // 512                # 8 column tiles
_EPS = 1e-6
_MARGIN = 1.0

_PROGS = {}  # compiled Bass programs keyed by reps


def _build_program(reps=1):
    """Build the SPMD program. reps>1 repeats the whole body (including
    input DMA) for wall-clock timing amplification in test.py."""
    import concourse.bacc as bacc
    import concourse.tile as tile
    from concourse import mybir

    f32 = mybir.dt.float32
    AF = mybir.ActivationFunctionType
    ALU = mybir.AluOpType
    AX = mybir.AxisListType

    nc = bacc.Bacc(
        "TRN2",
        target_bir_lowering=False,
        debug=False,
        enable_asserts=True,
        num_devices=_NCORES,
    )

    zT = nc.dram_tensor("zT", [_D, _N], f32, kind="ExternalInput").ap()
    zlocT = nc.dram_tensor("zlocT", [_D, _ROWS], f32, kind="ExternalInput").ap()
    zloc = nc.dram_tensor("zloc", [_ROWS, _D], f32, kind="ExternalInput").ap()
    lab_lhsT = nc.dram_tensor("lab_lhsT", [3, _ROWS], f32, kind="ExternalInput").ap()
    lab_rhs = nc.dram_tensor("lab_rhs", [3, _N], f32, kind="ExternalInput").ap()
    cls_rhs = nc.dram_tensor("cls_rhs", [3, _C], f32, kind="ExternalInput").ap()
    preds = nc.dram_tensor("preds", [_ROWS, _C], f32, kind="ExternalInput").ap()
    out_pair = nc.dram_tensor("out_pair", [128, _RB * _CT], f32, kind="ExternalOutput").ap()
    out_ce = nc.dram_tensor("out_ce", [128, _RB], f32, kind="ExternalOutput").ap()

    def emit(tc):
        with tc.tile_pool(name="const", bufs=1) as cpool, \
             tc.tile_pool(name="acc", bufs=1) as apool:
            # ---- resident inputs -------------------------------------------------
            zTa = cpool.tile([128, _N], f32)
            nc.sync.dma_start(zTa[:], zT[0:128, :])
            zTb = cpool.tile([128, _N], f32)
            nc.sync.dma_start(zTb[:], zT[128:256, :])
            zlTa = cpool.tile([128, _ROWS], f32)
            nc.sync.dma_start(zlTa[:], zlocT[0:128, :])
            zlTb = cpool.tile([128, _ROWS], f32)
            nc.sync.dma_start(zlTb[:], zlocT[128:256, :])
            lab_lhsT_sb = cpool.tile([3, _ROWS], f32)
            nc.sync.dma_start(lab_lhsT_sb[:], lab_lhsT[:])
            lab_rhs_sb = cpool.tile([3, _N], f32)
            nc.sync.dma_start(lab_rhs_sb[:], lab_rhs[:])
            cls_rhs_sb = cpool.tile([3, _C], f32)
            nc.sync.dma_start(cls_rhs_sb[:], cls_rhs[:])
            ones_row = cpool.tile([1, 128], f32)
            nc.vector.memset(ones_row[:], 1.0)
            ones_col = cpool.tile([128, 1], f32)
            nc.vector.memset(ones_col[:], 1.0)
            c_sb = cpool.tile([1, _N], f32)      # sq_n - 2 eps s_n + D eps^2
            b_sb = cpool.tile([128, _RB], f32)   # sq_m + 2 eps s_m, col per row tile
            pair_acc = apool.tile([128, _RB * _CT], f32)
            ce_acc = apool.tile([128, _RB], f32)

            # ---- preamble: column stats c_n and per-row bias b_m ----------------
            with tc.tile_pool(name="pre", bufs=2) as pre, \
                 tc.tile_pool(name="prepsum", bufs=1, space="PSUM") as prepsum:
                zT2a = pre.tile([128, _N], f32, tag="zt2a", bufs=1)
                nc.scalar.activation(zT2a[:], zTa[:], AF.Square)
                zT2b = pre.tile([128, _N], f32, tag="zt2b", bufs=1)
                nc.scalar.activation(zT2b[:], zTb[:], AF.Square)
                for ct in range(_CT):
                    cs = slice(ct * 512, (ct + 1) * 512)
                    ps_sq = prepsum.tile([1, 512], f32, tag="ps_sq", bufs=2)
                    nc.tensor.matmul(ps_sq[:], ones_col[:, 0:1], zT2a[:, cs],
                                     start=True, stop=False)
                    nc.tensor.matmul(ps_sq[:], ones_col[:, 0:1], zT2b[:, cs],
                                     start=False, stop=True)
                    ps_s = prepsum.tile([1, 512], f32, tag="ps_s", bufs=2)
                    nc.tensor.matmul(ps_s[:], ones_col[:, 0:1], zTa[:, cs],
                                     start=True, stop=False)
                    nc.tensor.matmul(ps_s[:], ones_col[:, 0:1], zTb[:, cs],
                                     start=False, stop=True)
                    # c = sq + (-2 eps) * s + D eps^2
                    nc.scalar.activation(c_sb[0:1, cs], ps_s[:], AF.Copy,
                                         bias=float(_D) * _EPS * _EPS,
                                         scale=-2.0 * _EPS)
                    nc.vector.tensor_add(c_sb[0:1, cs], c_sb[0:1, cs], ps_sq[:])
                for rb in range(_RB):
                    rs = slice(rb * 128, (rb + 1) * 128)
                    zl = pre.tile([128, _D], f32, tag="zl")
                    nc.sync.dma_start(zl[:], zloc[rs, :])
                    z2 = pre.tile([128, _D], f32, tag="z2")
                    nc.scalar.activation(z2[:], zl[:], AF.Square)
                    sq_r = pre.tile([128, 1], f32, tag="sq_r")
                    nc.vector.reduce_sum(sq_r[:], z2[:], axis=AX.X)
                    s_r = pre.tile([128, 1], f32, tag="s_r")
                    nc.vector.reduce_sum(s_r[:], zl[:], axis=AX.X)
                    s_sc = pre.tile([128, 1], f32, tag="s_sc")
                    nc.vector.tensor_scalar_mul(s_sc[:], s_r[:], 2.0 * _EPS)
                    nc.vector.tensor_add(b_sb[:, rb:rb + 1], sq_r[:], s_sc[:])

            # ---- main loops ------------------------------------------------------
            with tc.tile_pool(name="work", bufs=3) as wpool, \
                 tc.tile_pool(name="mpsum", bufs=2, space="PSUM") as mpsum:
                # CE over this core's preds rows
                for rb in range(_RB):
                    rs = slice(rb * 128, (rb + 1) * 128)
                    p_t = wpool.tile([128, _C], f32, tag="p")
                    nc.sync.dma_start(p_t[:], preds[rs, :])
                    nmx = wpool.tile([128, 1], f32, tag="nmx")
                    nc.vector.tensor_reduce(nmx[:], p_t[:], axis=AX.X,
                                            op=ALU.max, negate=True)
                    e_t = wpool.tile([128, _C], f32, tag="e")
                    se = wpool.tile([128, 1], f32, tag="se")
                    nc.scalar.activation(e_t[:], p_t[:], AF.Exp,
                                         bias=nmx[:, 0:1], scale=1.0,
                                         accum_out=se[:, 0:1])
                    l_t = wpool.tile([128, 1], f32, tag="l")
                    nc.scalar.activation(l_t[:], se[:], AF.Ln)
                    qc = mpsum.tile([128, _C], f32, tag="qc")
                    nc.tensor.matmul(qc[:, 0:512], lab_lhsT_sb[:, rs],
                                     cls_rhs_sb[:, 0:512], start=True, stop=True)
                    nc.tensor.matmul(qc[:, 512:_C], lab_lhsT_sb[:, rs],
                                     cls_rhs_sb[:, 512:_C], start=True, stop=True)
                    cm = wpool.tile([128, _C], f32, tag="cm")
                    nc.scalar.activation(cm[:], qc[:], AF.Relu,
                                         bias=1.0, scale=-1.0)
                    scr_ce = wpool.tile([128, _C], f32, tag="scr_ce")
                    plab = wpool.tile([128, 1], f32, tag="plab")
                    nc.vector.tensor_mul(scr_ce[:], p_t[:], cm[:])
                    nc.vector.reduce_sum(plab[:], scr_ce[:], axis=AX.X)
                    # ce_row = rowmax + log(sumexp) - p[label] = l - nmx - plab
                    t1 = wpool.tile([128, 1], f32, tag="t1")
                    nc.vector.tensor_sub(t1[:], l_t[:], nmx[:])
                    nc.vector.tensor_sub(ce_acc[:, rb:rb + 1], t1[:], plab[:])

                # pairwise block-row
                for rb in range(_RB):
                    rs = slice(rb * 128, (rb + 1) * 128)
                    for ct in range(_CT):
                        cs = slice(ct * 512, (ct + 1) * 512)
                        pp = mpsum.tile([128, 512], f32, tag="pp")
                        nc.tensor.matmul(pp[:], zlTa[:, rs], zTa[:, cs],
                                         start=True, stop=False)
                        nc.tensor.matmul(pp[:], zlTb[:, rs], zTb[:, cs],
                                         start=False, stop=False)
                        nc.tensor.matmul(pp[:], ones_row[0:1, :], c_sb[0:1, cs],
                                         start=False, stop=True)
                        qq = mpsum.tile([128, 512], f32, tag="qq")
                        nc.tensor.matmul(qq[:], lab_lhsT_sb[:, rs],
                                         lab_rhs_sb[:, cs], start=True, stop=True)
                        # r = relu(P + b) = clamped d2  (zlocT pre-scaled by -2)
                        r_t = wpool.tile([128, 512], f32, tag="r")
                        nc.scalar.activation(r_t[:], pp[:], AF.Relu,
                                             bias=b_sb[:, rb:rb + 1], scale=1.0)
                        d_t = wpool.tile([128, 512], f32, tag="d")
                        nc.scalar.activation(d_t[:], r_t[:], AF.Sqrt)
                        h_t = wpool.tile([128, 512], f32, tag="h")
                        nc.scalar.activation(h_t[:], d_t[:], AF.Relu,
                                             bias=_MARGIN, scale=-1.0)
                        # labels differ (q != 0) -> take dist; same -> hinge
                        # (mask must be integer-typed; f32 q==+0.0 iff same)
                        nc.vector.copy_predicated(
                            h_t[:], qq[:].bitcast(mybir.dt.int32), d_t[:])
                        scr = wpool.tile([128, 512], f32, tag="scr")
                        idx = rb * _CT + ct
                        nc.vector.tensor_mul(scr[:], h_t[:], h_t[:])
                        nc.vector.reduce_sum(pair_acc[:, idx:idx + 1], scr[:],
                                             axis=AX.X)

            nc.sync.dma_start(out_pair[:], pair_acc[:])
            nc.sync.dma_start(out_ce[:], ce_acc[:])

    with tile.TileContext(nc) as tc:
        for _rep in range(reps):
            emit(tc)

    nc.compile()
    return nc


def _get_program(reps=1):
    if reps not in _PROGS:
        _PROGS[reps] = _build_program(reps)
    return _PROGS[reps]


def kernel(latents, labels, preds):
    from concourse.bass_utils import run_bass_kernel_spmd

    lat = np.ascontiguousarray(np.asarray(latents, dtype=np.float32))
    lab = np.asarray(labels).astype(np.int64)
    prd = np.ascontiguousarray(np.asarray(preds, dtype=np.float32))
    assert lat.shape == (_N, _D) and prd.shape == (_N, _C) and lab.shape == (_N,)

    zT_full = np.ascontiguousarray(lat.T)                      # [D, N]
    labf = lab.astype(np.float32)                              # exact (< 2^24)
    lab2 = labf * labf
    onesN = np.ones(_N, np.float32)
    lab_rhs = np.ascontiguousarray(np.stack([onesN, -2.0 * labf, lab2]))
    iot = np.arange(_C, dtype=np.float32)
    cls_rhs = np.ascontiguousarray(
        np.stack([np.ones(_C, np.float32), -2.0 * iot, iot * iot]))

    in_maps = []
    for c in range(_NCORES):
        sl = slice(c * _ROWS, (c + 1) * _ROWS)
        in_maps.append({
            "zT": zT_full,
            # pre-scale by -2 so the gram matmuls produce -2*G directly
            "zlocT": np.ascontiguousarray(-2.0 * zT_full[:, sl]),
            "zloc": np.ascontiguousarray(lat[sl]),
            "lab_lhsT": np.ascontiguousarray(
                np.stack([lab2[sl], labf[sl], np.ones(_ROWS, np.float32)])),
            "lab_rhs": lab_rhs,
            "cls_rhs": cls_rhs,
            "preds": np.ascontiguousarray(prd[sl]),
        })

    nc = _get_program()
    res = run_bass_kernel_spmd(nc, in_maps, core_ids=list(range(_NCORES)))

    pair_sum = 0.0
    ce_sum = 0.0
    for r in res.results:
        pair_sum += float(r["out_pair"].astype(np.float64).sum())
        ce_sum += float(r["out_ce"].astype(np.float64).sum())

    # diagonal of the full-matrix sum: d2_ii = max(D*eps^2, 1e-12), same label
    d2ii = max(_D * _EPS * _EPS, 1e-12)
    hii = max(_MARGIN - np.sqrt(d2ii), 0.0)
    diag = _N * hii * hii
    contrastive = (pair_sum - diag) / (_N * (_N - 1.0))
    supervised = ce_sum / _N
    total = contrastive + supervised
    return (np.float32(total), np.float32(contrastive), np.float32(supervised))
